# revision 8
# baseline (speedup 1.0000x reference)
"""Cross-Scale Non-Local Attention kernel for 8x Trainium2 NeuronCores.

Data-parallel over batch: each of the 8 cores processes one sample
(B=8, H=W=64, C=64). Per-core Bass/Tile program:

  1. x loaded in 4 chunks; PE-transposed to channel-major xT [64, 4096];
     bilinear partials accumulated per chunk on DVE.
  2. phi computed 4-fold replicated on 128 partitions (phi_w stacked 4x)
     so the 3x3 patch taps can be packed 4-per-matmul: stationaries
     phi_pack [128=4taps*32ci, 2grp, 256n] in fp16; L2 norms ->
     s10 = 10/max(norm,1e-6) transposed to [128,2] on the PE.
  3. theta computed 4-fold replicated (theta_w stacked 4x), prelu on
     [128,512]; gpsimd builds per-tap-shifted bf16 views Th_g0/Th_g1
     [128, 64, 64] and Th1 [32, 64, 64] so score matmuls use K=128.
  4. g = prelu(xT.T @ g_w)/6 in bf16 (batched prelu on [128,512]),
     written into the interior of a zero-padded DRAM image g_pad
     [72,72,64]; 18 shifted dynamic-filter views kg[q,qw,kb]
     [n=128, (r rw c)=1024] gathered back by strided DMA.
  5. Per pixel-chunk ch: scoresT [n,pix] via 2 K=128 + 1 K=32 matmuls
     (fp16 phi stationary, bf16 theta moving); E = exp(s10*score);
     S = ones.T @ E; E *= 1/S broadcast.
  6. Deconv as polyphase conv-transpose, one chunk behind the scores:
     psum[rc 128, pix 512] += kg[q,qw,kb][:, u*128:(u+1)*128].T @ E_win
     over 18 shifts; PSUM drained to bf16, PE-transposed in bf16, cast
     to f32, one batched DMA pair per (pc, u).
"""

import numpy as np

_CACHE = {}

B = 8
H = W = 64
C = 64
CI = 32
HS = WS = 16
N = 256
PH = 66          # padded attn spatial extent (64 + 1 halo each side)
GP = 72          # padded g spatial extent (64 + 4 each side)

# tap packing groups: G0 = taps 0-3, G1 = taps 4-7, tap 8 = (2,2) alone
TAPS = [(kh, kw) for kh in range(3) for kw in range(3)]


def _build_nc():
    import concourse.bass as bass
    import concourse.tile as tile
    from concourse import bacc, mybir
    from concourse.masks import make_identity
    from contextlib import ExitStack

    F32 = mybir.dt.float32
    F32R = mybir.dt.float32r
    BF16 = mybir.dt.bfloat16
    F16 = mybir.dt.float16
    Alu = mybir.AluOpType
    Act = mybir.ActivationFunctionType

    def r_(ap):
        return ap.bitcast(F32R)

    nc = bacc.Bacc("TRN2", debug=False)

    x_h = nc.dram_tensor("x", [H, W, C], F32, kind="ExternalInput")
    thw_h = nc.dram_tensor("theta_w", [C, CI], F32, kind="ExternalInput")
    thb_h = nc.dram_tensor("theta_b", [CI], F32, kind="ExternalInput")
    tha_h = nc.dram_tensor("theta_alpha", [CI], F32, kind="ExternalInput")
    phw_h = nc.dram_tensor("phi_w", [C, CI], F32, kind="ExternalInput")
    phb_h = nc.dram_tensor("phi_b", [CI], F32, kind="ExternalInput")
    pha_h = nc.dram_tensor("phi_alpha", [CI], F32, kind="ExternalInput")
    gw_h = nc.dram_tensor("g_w", [C, C], F32, kind="ExternalInput")
    gb_h = nc.dram_tensor("g_b", [C], F32, kind="ExternalInput")
    ga_h = nc.dram_tensor("g_alpha", [C], F32, kind="ExternalInput")
    y_h = nc.dram_tensor("y", [4 * H, 4 * W, C], F32, kind="ExternalOutput")

    with tile.TileContext(nc) as tc, ExitStack() as top:
        ec = top.enter_context

        consts = ec(tc.tile_pool(name="consts", bufs=1))
        xp_pool = ec(tc.tile_pool(name="xp_pool", bufs=1))
        persist = ec(tc.tile_pool(name="persist", bufs=1))
        phip = ec(tc.tile_pool(name="phip", bufs=1))
        dramp = ec(tc.tile_pool(name="dramp", bufs=1, space="DRAM"))
        staging = ec(tc.tile_pool(name="staging", bufs=3))
        ps_misc = ec(tc.tile_pool(name="ps_misc", bufs=2, space="PSUM"))
        ps_sc = ec(tc.tile_pool(name="ps_sc", bufs=2, space="PSUM"))
        ps_d = ec(tc.tile_pool(name="ps_d", bufs=2, space="PSUM"))
        ps_tr = ec(tc.tile_pool(name="ps_tr", bufs=2, space="PSUM"))

        # ---- constants / weights ----
        ident = consts.tile([128, 128], F32)
        make_identity(nc, ident)
        identB = consts.tile([128, 128], BF16)
        nc.vector.tensor_copy(out=identB, in_=ident)
        # HAM warmup + keepalive matmuls
        wu = consts.tile([128, 512], F32)
        nc.vector.memset(wu, 0.0)
        ps_wu = ps_sc.tile([128, 512], F32, tag="sc", name="ps_wu")
        for i in range(8):
            nc.tensor.matmul(ps_wu, wu[:, :128], wu, start=True, stop=True)

        def keepalive(tag):
            ps_ka = ps_sc.tile([128, 512], F32, tag="sc", name=f"ka{tag}")
            nc.tensor.matmul(ps_ka, wu[:, :128], wu, start=True, stop=True)

        # x chunks first on the DMA queues
        xP = xp_pool.tile([128, 32, C], F32)
        x_r = x_h.ap().rearrange("h w c -> (h w) c").rearrange(
            "(t p) c -> p t c", p=128)
        for xc in range(4):
            nc.sync.dma_start(
                out=xP[:, xc * 8:(xc + 1) * 8, :],
                in_=x_r[:, xc * 8:(xc + 1) * 8, :])
        # 4x stacked weights for replicated theta/phi
        thw4 = consts.tile([C, 128], F32)
        phw4 = consts.tile([C, 128], F32)
        thb4 = consts.tile([128, 1], F32)
        tha4 = consts.tile([128, 1], F32)
        phb4 = consts.tile([128, 1], F32)
        pha4 = consts.tile([128, 1], F32)
        for b4 in range(4):
            sl = slice(b4 * CI, (b4 + 1) * CI)
            nc.sync.dma_start(out=r_(thw4[:, sl]), in_=r_(thw_h.ap()))
            nc.sync.dma_start(out=r_(phw4[:, sl]), in_=r_(phw_h.ap()))
            nc.sync.dma_start(out=thb4[sl], in_=thb_h.ap().unsqueeze(1))
            nc.sync.dma_start(out=tha4[sl], in_=tha_h.ap().unsqueeze(1))
            nc.sync.dma_start(out=phb4[sl], in_=phb_h.ap().unsqueeze(1))
            nc.sync.dma_start(out=pha4[sl], in_=pha_h.ap().unsqueeze(1))
        gw_sb = consts.tile([C, C], F32)
        nc.sync.dma_start(out=r_(gw_sb), in_=r_(gw_h.ap()))
        gb_row = consts.tile([1, C], F32)
        nc.sync.dma_start(out=gb_row, in_=gb_h.ap().unsqueeze(0))
        ga_row = consts.tile([1, C], F32)
        nc.sync.dma_start(out=ga_row, in_=ga_h.ap().unsqueeze(0))
        gb8_row = consts.tile([1, 8, C], F32)
        ga8_row = consts.tile([1, 8, C], F32)
        for i in range(8):
            nc.vector.tensor_copy(out=gb8_row[:, i, :], in_=gb_row)
            nc.vector.tensor_copy(out=ga8_row[:, i, :], in_=ga_row)
        gb8_bc = consts.tile([128, 512], F32)
        nc.gpsimd.partition_broadcast(
            gb8_bc, gb8_row.rearrange("p a b -> p (a b)"))
        ga6_8bc = consts.tile([128, 512], F32)
        nc.gpsimd.partition_broadcast(
            ga6_8bc, ga8_row.rearrange("p a b -> p (a b)"))
        nc.vector.tensor_scalar_mul(ga6_8bc, ga6_8bc, 1.0 / 6.0)
        z66 = consts.tile([128, PH], F32)
        nc.vector.memset(z66, 0.0)
        o1 = consts.tile([128, 1], F32)
        nc.vector.memset(o1, 1.0)
        ones32 = consts.tile([CI, 1], F32)
        nc.vector.tensor_copy(out=r_(ones32), in_=o1[:CI])
        ones128 = consts.tile([128, 1], BF16)
        nc.vector.tensor_copy(out=ones128, in_=o1)
        s10T = consts.tile([128, 2], F32)

        # ---- persistent activation buffers ----
        # per-tap-shifted theta views (bf16 moving operands, K-packed)
        Th_g0 = persist.tile([128, H, W], BF16)
        Th_g1 = persist.tile([128, H, W], BF16)
        Th_1 = persist.tile([CI, H, W], BF16)
        nc.vector.memset(Th_g0, 0.0)
        nc.vector.memset(Th_g1, 0.0)
        nc.vector.memset(Th_1, 0.0)
        theta4 = persist.tile([128, H, W], BF16)   # unshifted, 4x replicated
        attnT = persist.tile([128, 2, PH, PH], BF16)
        for kb in range(2):
            nc.vector.tensor_copy(out=attnT[:, kb, 0, :], in_=z66)
            nc.vector.tensor_copy(out=attnT[:, kb, PH - 1, :], in_=z66)
            nc.vector.tensor_copy(out=attnT[:, kb, :, 0], in_=z66)
            nc.vector.tensor_copy(out=attnT[:, kb, :, PH - 1], in_=z66)
        phi_pack = persist.tile([128, 2, N], F16)  # grp 0/1 stationaries
        phi22 = persist.tile([CI, N], F16)         # tap (2,2) stationary

        phiT_pad4 = phip.tile([128, 18, 18], F32)
        nc.vector.memset(phiT_pad4, 0.0)
        n2p = phip.tile([1, 324], F32)
        nrm = phip.tile([1, N], F32)
        phi_inT = phip.tile([C, HS, WS], F32)

        # zero-padded g image in DRAM; polyphase kg views are strided reads
        g_pad = dramp.tile([GP, GP, C], BF16)
        ztb = consts.tile([128, 512], BF16)
        nc.vector.memset(ztb, 0.0)
        gpf = g_pad.rearrange("a b c -> (a b c)")
        head = gpf[: 5 * 65536].rearrange("(k p f) -> k p f", p=128, f=512)
        for k in range(5):
            nc.sync.dma_start(out=head[k], in_=ztb)
        tail = gpf[5 * 65536:].rearrange("(p f) -> p f", f=512)
        nc.sync.dma_start(out=tail, in_=ztb[: tail.shape[0], :])

        with ExitStack() as st1:
            e1 = st1.enter_context
            xt_pool = e1(tc.tile_pool(name="xt_pool", bufs=1))
            gsb_pool = e1(tc.tile_pool(name="gsb_pool", bufs=1))
            ttmp = e1(tc.tile_pool(name="ttmp", bufs=2))
            gtmp = e1(tc.tile_pool(name="gtmp", bufs=3))

            xT = xt_pool.tile([C, H, W], F32)
            xTf = xT.rearrange("c h w -> c (h w)")
            g_sb = gsb_pool.tile([128, 32, C], BF16)
            xv = xT.rearrange("c (hq hs) (wq ws) -> c hq hs wq ws",
                              hs=4, ws=4)

            # -- x transposes + bilinear partials per chunk --
            for xc in range(4):
                for t in range(xc * 8, (xc + 1) * 8):
                    ps_x = ps_misc.tile([C, 128], F32, tag="m",
                                        name=f"ps_x{t}")
                    nc.tensor.transpose(ps_x, xP[:, t, :], ident)
                    nc.scalar.copy(
                        out=r_(xTf[:, t * 128:(t + 1) * 128]), in_=ps_x)
                xvc = xv[:, 4 * xc:4 * (xc + 1)]
                pslc = phi_inT[:, 4 * xc:4 * (xc + 1), :]
                nc.vector.tensor_add(r_(pslc), xvc[:, :, 1, :, 1],
                                     xvc[:, :, 1, :, 2])
                nc.vector.tensor_add(r_(pslc), pslc, xvc[:, :, 2, :, 1])
                nc.vector.tensor_add(r_(pslc), pslc, xvc[:, :, 2, :, 2])
                nc.vector.tensor_scalar_mul(r_(pslc), pslc, 0.25)
                keepalive(f"x{xc}")

            # -- phi (4x replicated) + patches + norms + s10 --
            ps_phi = ps_misc.tile([128, N], F32, tag="m")
            nc.tensor.matmul(
                ps_phi, r_(phw4), r_(phi_inT.rearrange("c a b -> c (a b)")),
                start=True, stop=True)
            p_lin = ttmp.tile([128, HS, WS], F32, tag="pl")
            nc.vector.tensor_scalar_add(
                p_lin.rearrange("p a b -> p (a b)"), ps_phi, phb4)
            p_neg = ttmp.tile([128, HS, WS], F32, tag="pn")
            nc.vector.tensor_scalar(
                p_neg.rearrange("p a b -> p (a b)"),
                p_lin.rearrange("p a b -> p (a b)"),
                0.0, pha4, Alu.min, Alu.mult)
            nc.vector.scalar_tensor_tensor(
                out=phiT_pad4[:, 1:17, 1:17],
                in0=p_lin, scalar=0.0, in1=p_neg,
                op0=Alu.max, op1=Alu.add)

            # patch stationaries: phi_pack[32b+ci, grp, n] = tap G[grp][b]
            for grp in range(2):
                for b4 in range(4):
                    kh, kw = TAPS[grp * 4 + b4]
                    sl = slice(b4 * CI, (b4 + 1) * CI)
                    nc.gpsimd.tensor_copy(
                        out=phi_pack[sl, grp, :].rearrange(
                            "p (a b) -> p a b", b=WS),
                        in_=phiT_pad4[sl, kh:kh + 16, kw:kw + 16])
            nc.gpsimd.tensor_copy(
                out=phi22.rearrange("p (a b) -> p a b", b=WS),
                in_=phiT_pad4[:CI, 2:18, 2:18])

            sq = ttmp.tile([CI, 324], F32, tag="sq")
            nc.scalar.activation(r_(sq),
                                 phiT_pad4[:CI].rearrange("p a b -> p (a b)"),
                                 Act.Square)
            ps_n2 = ps_misc.tile([1, 324], F32, tag="m")
            nc.tensor.matmul(ps_n2, r_(ones32), r_(sq), start=True, stop=True)
            nc.scalar.copy(out=n2p, in_=ps_n2)
            n2v = n2p.rearrange("p (a b) -> p a b", b=18)
            nrm3 = nrm.rearrange("p (a b) -> p a b", b=WS)
            nc.vector.tensor_add(nrm3, n2v[:, 0:16, 0:16], n2v[:, 0:16, 1:17])
            for kh in range(3):
                for kw in range(3):
                    if kh == 0 and kw < 2:
                        continue
                    nc.vector.tensor_add(
                        nrm3, nrm3, n2v[:, kh:kh + 16, kw:kw + 16])
            nc.scalar.sqrt(nrm, nrm)
            nc.vector.tensor_scalar_max(nrm, nrm, 1e-6)
            nc.vector.reciprocal(nrm, nrm)
            nc.vector.tensor_scalar_mul(nrm, nrm, 10.0)
            ps_s10 = ps_misc.tile([128, 2], F32, tag="m", name="ps_s10")
            for kb in range(2):
                nc.tensor.transpose(
                    ps_s10[:, kb:kb + 1],
                    nrm[:, kb * 128:(kb + 1) * 128], ident[:1, :1])
            nc.scalar.copy(out=s10T, in_=ps_s10)

            # -- theta chunks: 4x replicated matmul + prelu + shifted views --
            def theta_chunk(c):
                h0 = c * 8
                ps_t = ps_misc.tile([128, 512], F32, tag="m",
                                    name=f"ps_t{c}")
                nc.tensor.matmul(
                    ps_t, r_(thw4), r_(xTf[:, c * 512:(c + 1) * 512]),
                    start=True, stop=True)
                t_lin = ttmp.tile([128, 8, W], F32, tag="tl")
                nc.vector.tensor_scalar_add(
                    t_lin.rearrange("p a b -> p (a b)"), ps_t, thb4)
                t_neg = ttmp.tile([128, 8, W], F32, tag="tn")
                nc.vector.tensor_scalar(
                    t_neg.rearrange("p a b -> p (a b)"),
                    t_lin.rearrange("p a b -> p (a b)"),
                    0.0, tha4, Alu.min, Alu.mult)
                nc.vector.scalar_tensor_tensor(
                    out=theta4[:, h0:h0 + 8, :], in0=t_lin, scalar=0.0,
                    in1=t_neg, op0=Alu.max, op1=Alu.add)
                # shifted views on gpsimd (lane-aligned partition blocks)
                for grp in range(2):
                    dst = Th_g0 if grp == 0 else Th_g1
                    for b4 in range(4):
                        kh, kw = TAPS[grp * 4 + b4]
                        sl = slice(b4 * CI, (b4 + 1) * CI)
                        hlo = max(0, h0 + 1 - kh)
                        hhi = min(H, h0 + 9 - kh)
                        if hhi <= hlo:
                            continue
                        wlo = max(0, 1 - kw)
                        whi = min(W, W + 1 - kw)
                        nc.gpsimd.tensor_copy(
                            out=dst[sl, hlo:hhi, wlo:whi],
                            in_=theta4[sl, hlo + kh - 1:hhi + kh - 1,
                                       wlo + kw - 1:whi + kw - 1])
                hlo = max(0, h0 - 1)
                hhi = min(H, h0 + 7)
                if hhi > hlo:
                    nc.gpsimd.tensor_copy(
                        out=Th_1[:, hlo:hhi, 0:63],
                        in_=theta4[:CI, hlo + 1:hhi + 1, 1:64])

            for c in range(4):
                theta_chunk(c)
            keepalive("t03")

            # -- g path: batched matmuls + prelu per chunk, then DRAM --
            for xc in range(4):
                ps_g8 = ps_misc.tile([128, 512], F32, tag="m",
                                     name=f"ps_g8_{xc}")
                for i, t in enumerate(range(xc * 8, (xc + 1) * 8)):
                    nc.tensor.matmul(
                        ps_g8[:, i * 64:(i + 1) * 64],
                        r_(xTf[:, t * 128:(t + 1) * 128]), r_(gw_sb),
                        start=True, stop=True)
                gv = gtmp.tile([128, 512], F32, tag="gv")
                nc.vector.tensor_add(gv, ps_g8, gb8_bc)
                gm1 = gtmp.tile([128, 512], F32, tag="gm1")
                nc.vector.tensor_scalar_max(gm1, gv, 0.0)
                nc.vector.tensor_scalar_min(gv, gv, 0.0)
                nc.vector.tensor_mul(gv, gv, ga6_8bc)
                nc.vector.scalar_tensor_tensor(
                    out=g_sb[:, xc * 8:(xc + 1) * 8, :].rearrange(
                        "p a b -> p (a b)"),
                    in0=gm1, scalar=1.0 / 6.0, in1=gv,
                    op0=Alu.mult, op1=Alu.add)
            g_int = g_pad[4:68, 4:68, :].rearrange(
                "(t a) w c -> a w t c", a=2)
            for p1 in range(2):
                nc.sync.dma_start(
                    out=g_int[p1], in_=g_sb[p1 * 64:(p1 + 1) * 64, :, :])
            keepalive("g")

            for c in range(4, 8):
                theta_chunk(c)

        # ---- stage 2: fused scores/softmax/deconv pipeline ----
        with ExitStack() as st2:
            e2 = st2.enter_context
            kgp = e2(tc.tile_pool(name="kgp", bufs=1))
            rbp = e2(tc.tile_pool(name="rbp", bufs=3))
            schp = e2(tc.tile_pool(name="schp", bufs=2))
            trp = e2(tc.tile_pool(name="trp", bufs=3))

            # gather the 18 dynamic-filter tiles straight from g_pad
            gp4 = g_pad.rearrange("(hq hr) (wq wr) c -> hq wq hr (wr c)",
                                  hr=4, wr=4)
            kg = {}
            for q in range(3):
                for qw in range(3):
                    for kb in range(2):
                        t_ = kgp.tile([128, 4, 256], BF16,
                                      tag=f"kg{q}{qw}{kb}",
                                      name=f"kg{q}{qw}{kb}")
                        for r in range(4):
                            nc.sync.dma_start(
                                out=t_[:, r, :],
                                in_=gp4[kb * 8 + q: kb * 8 + q + 8,
                                        qw: qw + 16, r, :])
                        kg[(q, qw, kb)] = t_.rearrange("p r x -> p (r x)")

            yr3 = y_h.ap().rearrange(
                "(pc kk pp r) (Mw w) c -> pc pp Mw kk r w c",
                kk=4, pp=2, r=4, w=4)
            pending = [None]

            def drain(pend):
                tr_in, pc, u = pend
                ps_t2 = ps_tr.tile([128, 512], BF16, tag="tt",
                                   name=f"ps_tr{pc}_{u}")
                for k in range(4):
                    nc.tensor.transpose(
                        ps_t2[:, k * 128:(k + 1) * 128],
                        tr_in[:, k * 128:(k + 1) * 128], identB)
                st_ = staging.tile([128, 512], F32, tag="stg",
                                   name=f"st{pc}_{u}")
                nc.scalar.copy(out=st_, in_=ps_t2)
                st3 = st_.rearrange("p (k rw c) -> p k rw c", k=4, rw=2)
                rr = u // 2
                w0 = 2 * (u % 2)
                for p1 in range(2):
                    nc.sync.dma_start(
                        out=yr3[pc, p1, :, :, rr, w0:w0 + 2, :],
                        in_=st3[p1 * 64:(p1 + 1) * 64])

            def deconv_pc(pc):
                h0 = pc * 8
                for u in range(8):
                    ps_o = ps_d.tile([128, 512], F32, tag="d",
                                     name=f"ps_o{pc}_{u}")
                    first = True
                    for q in range(3):
                        for qw in range(3):
                            for kb in range(2):
                                nc.tensor.matmul(
                                    ps_o,
                                    kg[(q, qw, kb)][:, u * 128:
                                                    (u + 1) * 128],
                                    attnT[:, kb, h0 + 2 - q:h0 + 10 - q,
                                          2 - qw:66 - qw],
                                    start=first,
                                    stop=(q == 2 and qw == 2 and kb == 1))
                                first = False
                    tr_in = trp.tile([128, 512], BF16, tag="ti",
                                     name=f"ti{pc}_{u}")
                    nc.scalar.copy(out=tr_in, in_=ps_o)
                    if pending[0] is not None:
                        drain(pending[0])
                    pending[0] = (tr_in, pc, u)

            for ch in range(8):
                h0 = ch * 8
                for kb in range(2):
                    ps_s = ps_sc.tile([128, 512], F32, tag="sc",
                                      name=f"ps_s{ch}_{kb}")
                    nc.tensor.matmul(
                        ps_s, phi_pack[:, 0, kb * 128:(kb + 1) * 128],
                        Th_g0[:, h0:h0 + 8, :],
                        start=True, stop=False)
                    nc.tensor.matmul(
                        ps_s, phi_pack[:, 1, kb * 128:(kb + 1) * 128],
                        Th_g1[:, h0:h0 + 8, :],
                        start=False, stop=False)
                    nc.tensor.matmul(
                        ps_s, phi22[:, kb * 128:(kb + 1) * 128],
                        Th_1[:, h0:h0 + 8, :],
                        start=False, stop=True)
                    nc.scalar.activation(
                        out=attnT[:, kb, 1 + h0:9 + h0, 1:65],
                        in_=ps_s.rearrange("p (a b) -> p a b", b=64),
                        func=Act.Exp, scale=s10T[:, kb:kb + 1])
                if ch >= 2:
                    deconv_pc(ch - 2)
                ps_S = ps_misc.tile([1, 512], F32, tag="m", name=f"ps_S{ch}")
                for kb in range(2):
                    nc.tensor.matmul(
                        ps_S, ones128,
                        attnT[:, kb, 1 + h0:9 + h0, 1:65],
                        start=(kb == 0), stop=(kb == 1))
                sch = schp.tile([1, 512], F32, tag="sch", name=f"sch{ch}")
                nc.vector.reciprocal(sch, ps_S)
                rb_t = rbp.tile([128, 512], F32, tag="rb", name=f"rb{ch}")
                nc.gpsimd.partition_broadcast(rb_t, sch)
                rb3 = rb_t.rearrange("p (a b) -> p a b", b=64)
                for kb in range(2):
                    nc.vector.tensor_mul(
                        attnT[:, kb, 1 + h0:9 + h0, 1:65],
                        attnT[:, kb, 1 + h0:9 + h0, 1:65], rb3)
            deconv_pc(6)
            deconv_pc(7)
            drain(pending[0])

    nc.finalize()
    return nc


def kernel(**inputs):
    from concourse.bass_utils import run_bass_kernel_spmd

    if "nc" not in _CACHE:
        _CACHE["nc"] = _build_nc()
    nc = _CACHE["nc"]

    arrs = {k: np.ascontiguousarray(np.asarray(v, dtype=np.float32))
            for k, v in inputs.items()}
    x = arrs.pop("x")
    in_maps = [dict(arrs, x=x[b]) for b in range(B)]
    res = run_bass_kernel_spmd(nc, in_maps, core_ids=list(range(B)))
    return np.stack([res.results[b]["y"] for b in range(B)])


# revision 9
# speedup vs baseline: 1.1824x; 1.1824x over previous
"""Cross-Scale Non-Local Attention kernel for 8x Trainium2 NeuronCores.

Data-parallel over batch: each of the 8 cores processes one sample
(B=8, H=W=64, C=64). Per-core Bass/Tile program:

  1. x loaded in 4 chunks; PE-transposed to channel-major xT [64, 4096];
     bilinear partials accumulated per chunk on DVE.
  2. phi computed 4-fold replicated on 128 partitions (phi_w stacked 4x)
     so the 3x3 patch taps can be packed 4-per-matmul: stationaries
     phi_pack [128=4taps*32ci, 2grp, 256n] in fp16; L2 norms ->
     s10 = 10/max(norm,1e-6) transposed to [128,2] on the PE.
  3. theta computed 4-fold replicated (theta_w stacked 4x), prelu on
     [128,512]; gpsimd builds per-tap-shifted bf16 views Th_g0/Th_g1
     [128, 64, 64] and Th1 [32, 64, 64] so score matmuls use K=128.
  4. g = prelu(xT.T @ g_w)/6 in bf16 (batched prelu on [128,512]),
     written into the interior of a zero-padded DRAM image g_pad
     [72,72,64]; 18 shifted dynamic-filter views kg[q,qw,kb]
     [n=128, (r rw c)=1024] gathered back by strided DMA.
  5. Per pixel-chunk ch: scoresT [n,pix] via 2 K=128 + 1 K=32 matmuls
     (fp16 phi stationary, bf16 theta moving); E = exp(s10*score);
     S = ones.T @ E; E *= 1/S broadcast.
  6. Deconv as polyphase conv-transpose, one chunk behind the scores:
     psum[rc 128, pix 512] += kg[q,qw,kb][:, u*128:(u+1)*128].T @ E_win
     over 18 shifts; PSUM drained to bf16, PE-transposed in bf16, cast
     to f32, one batched DMA pair per (pc, u).
"""

import numpy as np

_CACHE = {}

B = 8
H = W = 64
C = 64
CI = 32
HS = WS = 16
N = 256
PH = 66          # padded attn spatial extent (64 + 1 halo each side)
GP = 72          # padded g spatial extent (64 + 4 each side)

# tap packing groups: G0 = taps 0-3, G1 = taps 4-7, tap 8 = (2,2) alone
TAPS = [(kh, kw) for kh in range(3) for kw in range(3)]


def _build_nc():
    import concourse.bass as bass
    import concourse.tile as tile
    from concourse import bacc, mybir
    from concourse.masks import make_identity
    from contextlib import ExitStack

    F32 = mybir.dt.float32
    F32R = mybir.dt.float32r
    BF16 = mybir.dt.bfloat16
    F16 = mybir.dt.float16
    Alu = mybir.AluOpType
    Act = mybir.ActivationFunctionType

    def r_(ap):
        return ap.bitcast(F32R)

    nc = bacc.Bacc("TRN2", debug=False)

    x_h = nc.dram_tensor("x", [H, W, C], F32, kind="ExternalInput")
    thw_h = nc.dram_tensor("theta_w", [C, CI], F32, kind="ExternalInput")
    thb_h = nc.dram_tensor("theta_b", [CI], F32, kind="ExternalInput")
    tha_h = nc.dram_tensor("theta_alpha", [CI], F32, kind="ExternalInput")
    phw_h = nc.dram_tensor("phi_w", [C, CI], F32, kind="ExternalInput")
    phb_h = nc.dram_tensor("phi_b", [CI], F32, kind="ExternalInput")
    pha_h = nc.dram_tensor("phi_alpha", [CI], F32, kind="ExternalInput")
    gw_h = nc.dram_tensor("g_w", [C, C], F32, kind="ExternalInput")
    gb_h = nc.dram_tensor("g_b", [C], F32, kind="ExternalInput")
    ga_h = nc.dram_tensor("g_alpha", [C], F32, kind="ExternalInput")
    y_h = nc.dram_tensor("y", [4 * H, 4 * W, C], F32, kind="ExternalOutput")

    with tile.TileContext(nc) as tc, ExitStack() as top:
        ec = top.enter_context

        consts = ec(tc.tile_pool(name="consts", bufs=1))
        xp_pool = ec(tc.tile_pool(name="xp_pool", bufs=1))
        persist = ec(tc.tile_pool(name="persist", bufs=1))
        phip = ec(tc.tile_pool(name="phip", bufs=1))
        dramp = ec(tc.tile_pool(name="dramp", bufs=1, space="DRAM"))
        staging = ec(tc.tile_pool(name="staging", bufs=3))
        ps_misc = ec(tc.tile_pool(name="ps_misc", bufs=2, space="PSUM"))
        ps_sc = ec(tc.tile_pool(name="ps_sc", bufs=2, space="PSUM"))
        ps_d = ec(tc.tile_pool(name="ps_d", bufs=2, space="PSUM"))
        ps_tr = ec(tc.tile_pool(name="ps_tr", bufs=2, space="PSUM"))

        # ---- constants / weights ----
        ident = consts.tile([128, 128], F32)
        make_identity(nc, ident)
        identB = consts.tile([128, 128], BF16)
        nc.vector.tensor_copy(out=identB, in_=ident)
        # HAM warmup + keepalive matmuls
        wu = consts.tile([128, 512], F32)
        nc.vector.memset(wu, 0.0)
        ps_wu = ps_sc.tile([128, 512], F32, tag="sc", name="ps_wu")
        for i in range(8):
            nc.tensor.matmul(ps_wu, wu[:, :128], wu, start=True, stop=True)

        def keepalive(tag):
            ps_ka = ps_sc.tile([128, 512], F32, tag="sc", name=f"ka{tag}")
            nc.tensor.matmul(ps_ka, wu[:, :128], wu, start=True, stop=True)

        # x chunks first on the DMA queues
        xP = xp_pool.tile([128, 32, C], F32)
        x_r = x_h.ap().rearrange("h w c -> (h w) c").rearrange(
            "(t p) c -> p t c", p=128)
        for xc in range(4):
            nc.sync.dma_start(
                out=xP[:, xc * 8:(xc + 1) * 8, :],
                in_=x_r[:, xc * 8:(xc + 1) * 8, :])
        thw_sb = consts.tile([C, CI], F32)
        nc.sync.dma_start(out=r_(thw_sb), in_=r_(thw_h.ap()))
        phw_sb = consts.tile([C, CI], F32)
        nc.sync.dma_start(out=r_(phw_sb), in_=r_(phw_h.ap()))
        thb_sb = consts.tile([CI, 1], F32)
        nc.sync.dma_start(out=thb_sb, in_=thb_h.ap().unsqueeze(1))
        tha_sb = consts.tile([CI, 1], F32)
        nc.sync.dma_start(out=tha_sb, in_=tha_h.ap().unsqueeze(1))
        phb_sb = consts.tile([CI, 1], F32)
        nc.sync.dma_start(out=phb_sb, in_=phb_h.ap().unsqueeze(1))
        pha_sb = consts.tile([CI, 1], F32)
        nc.sync.dma_start(out=pha_sb, in_=pha_h.ap().unsqueeze(1))
        gw_sb = consts.tile([C, C], F32)
        nc.sync.dma_start(out=r_(gw_sb), in_=r_(gw_h.ap()))
        gb_row = consts.tile([1, C], F32)
        nc.sync.dma_start(out=gb_row, in_=gb_h.ap().unsqueeze(0))
        ga_row = consts.tile([1, C], F32)
        nc.sync.dma_start(out=ga_row, in_=ga_h.ap().unsqueeze(0))
        gb8_row = consts.tile([1, 8, C], F32)
        ga8_row = consts.tile([1, 8, C], F32)
        for i in range(8):
            nc.vector.tensor_copy(out=gb8_row[:, i, :], in_=gb_row)
            nc.vector.tensor_copy(out=ga8_row[:, i, :], in_=ga_row)
        gb8_bc = consts.tile([128, 512], F32)
        nc.gpsimd.partition_broadcast(
            gb8_bc, gb8_row.rearrange("p a b -> p (a b)"))
        ga6_8bc = consts.tile([128, 512], F32)
        nc.gpsimd.partition_broadcast(
            ga6_8bc, ga8_row.rearrange("p a b -> p (a b)"))
        nc.vector.tensor_scalar_mul(ga6_8bc, ga6_8bc, 1.0 / 6.0)
        z66 = consts.tile([128, PH], F32)
        nc.vector.memset(z66, 0.0)
        o1 = consts.tile([128, 1], F32)
        nc.vector.memset(o1, 1.0)
        ones32 = consts.tile([CI, 1], F32)
        nc.vector.tensor_copy(out=r_(ones32), in_=o1[:CI])
        ones128 = consts.tile([128, 1], BF16)
        nc.vector.tensor_copy(out=ones128, in_=o1)
        s10T = consts.tile([128, 2], F32)

        # ---- persistent activation buffers ----
        thetaT_pad = persist.tile([CI, PH, PH], BF16)
        nc.vector.tensor_copy(out=thetaT_pad[:, 0, :], in_=z66[:CI])
        nc.vector.tensor_copy(out=thetaT_pad[:, PH - 1, :], in_=z66[:CI])
        nc.vector.tensor_copy(out=thetaT_pad[:, :, 0], in_=z66[:CI])
        nc.vector.tensor_copy(out=thetaT_pad[:, :, PH - 1], in_=z66[:CI])
        attnT = persist.tile([128, 2, PH, PH], BF16)
        for kb in range(2):
            nc.vector.tensor_copy(out=attnT[:, kb, 0, :], in_=z66)
            nc.vector.tensor_copy(out=attnT[:, kb, PH - 1, :], in_=z66)
            nc.vector.tensor_copy(out=attnT[:, kb, :, 0], in_=z66)
            nc.vector.tensor_copy(out=attnT[:, kb, :, PH - 1], in_=z66)
        phi_patchT = persist.tile([CI, 3, 3, N], F16)

        phiT_pad = phip.tile([CI, 18, 18], F32)
        nc.vector.memset(phiT_pad, 0.0)
        n2p = phip.tile([1, 324], F32)
        nrm = phip.tile([1, N], F32)
        phi_inT = phip.tile([C, HS, WS], F32)

        # zero-padded g image in DRAM; polyphase kg views are strided reads
        g_pad = dramp.tile([GP, GP, C], BF16)
        ztb = consts.tile([128, 512], BF16)
        nc.vector.memset(ztb, 0.0)
        gpf = g_pad.rearrange("a b c -> (a b c)")
        head = gpf[: 5 * 65536].rearrange("(k p f) -> k p f", p=128, f=512)
        for k in range(5):
            nc.sync.dma_start(out=head[k], in_=ztb)
        tail = gpf[5 * 65536:].rearrange("(p f) -> p f", f=512)
        nc.sync.dma_start(out=tail, in_=ztb[: tail.shape[0], :])

        with ExitStack() as st1:
            e1 = st1.enter_context
            xt_pool = e1(tc.tile_pool(name="xt_pool", bufs=1))
            gsb_pool = e1(tc.tile_pool(name="gsb_pool", bufs=1))
            ttmp = e1(tc.tile_pool(name="ttmp", bufs=2))
            gtmp = e1(tc.tile_pool(name="gtmp", bufs=3))

            xT = xt_pool.tile([C, H, W], F32)
            xTf = xT.rearrange("c h w -> c (h w)")
            g_sb = gsb_pool.tile([128, 32, C], BF16)
            xv = xT.rearrange("c (hq hs) (wq ws) -> c hq hs wq ws",
                              hs=4, ws=4)

            # -- x transposes + bilinear partials per chunk --
            for xc in range(4):
                for t in range(xc * 8, (xc + 1) * 8):
                    ps_x = ps_misc.tile([C, 128], F32, tag="m",
                                        name=f"ps_x{t}")
                    nc.tensor.transpose(ps_x, xP[:, t, :], ident)
                    nc.scalar.copy(
                        out=r_(xTf[:, t * 128:(t + 1) * 128]), in_=ps_x)
                xvc = xv[:, 4 * xc:4 * (xc + 1)]
                pslc = phi_inT[:, 4 * xc:4 * (xc + 1), :]
                nc.vector.tensor_add(r_(pslc), xvc[:, :, 1, :, 1],
                                     xvc[:, :, 1, :, 2])
                nc.vector.tensor_add(r_(pslc), pslc, xvc[:, :, 2, :, 1])
                nc.vector.tensor_add(r_(pslc), pslc, xvc[:, :, 2, :, 2])
                nc.vector.tensor_scalar_mul(r_(pslc), pslc, 0.25)
                keepalive(f"x{xc}")

            # -- phi + patches + norms + s10 --
            ps_phi = ps_misc.tile([CI, N], F32, tag="m")
            nc.tensor.matmul(
                ps_phi, r_(phw_sb), r_(phi_inT.rearrange("c a b -> c (a b)")),
                start=True, stop=True)
            p_lin = ttmp.tile([CI, HS, WS], F32, tag="pl")
            nc.vector.tensor_scalar_add(
                p_lin.rearrange("p a b -> p (a b)"), ps_phi, phb_sb)
            p_neg = ttmp.tile([CI, HS, WS], F32, tag="pn")
            nc.vector.tensor_scalar(
                p_neg.rearrange("p a b -> p (a b)"),
                p_lin.rearrange("p a b -> p (a b)"),
                0.0, pha_sb, Alu.min, Alu.mult)
            nc.vector.scalar_tensor_tensor(
                out=phiT_pad[:, 1:17, 1:17],
                in0=p_lin, scalar=0.0, in1=p_neg,
                op0=Alu.max, op1=Alu.add)

            for kh in range(3):
                for kw in range(3):
                    nc.vector.tensor_copy(
                        out=phi_patchT[:, kh, kw, :].rearrange(
                            "p (a b) -> p a b", b=WS),
                        in_=phiT_pad[:, kh:kh + 16, kw:kw + 16])

            sq = ttmp.tile([CI, 324], F32, tag="sq")
            nc.scalar.activation(r_(sq),
                                 phiT_pad.rearrange("p a b -> p (a b)"),
                                 Act.Square)
            ps_n2 = ps_misc.tile([1, 324], F32, tag="m")
            nc.tensor.matmul(ps_n2, r_(ones32), r_(sq), start=True, stop=True)
            nc.scalar.copy(out=n2p, in_=ps_n2)
            n2v = n2p.rearrange("p (a b) -> p a b", b=18)
            nrm3 = nrm.rearrange("p (a b) -> p a b", b=WS)
            nc.vector.tensor_add(nrm3, n2v[:, 0:16, 0:16], n2v[:, 0:16, 1:17])
            for kh in range(3):
                for kw in range(3):
                    if kh == 0 and kw < 2:
                        continue
                    nc.vector.tensor_add(
                        nrm3, nrm3, n2v[:, kh:kh + 16, kw:kw + 16])
            nc.scalar.sqrt(nrm, nrm)
            nc.vector.tensor_scalar_max(nrm, nrm, 1e-6)
            nc.vector.reciprocal(nrm, nrm)
            nc.vector.tensor_scalar_mul(nrm, nrm, 10.0)
            ps_s10 = ps_misc.tile([128, 2], F32, tag="m", name="ps_s10")
            for kb in range(2):
                nc.tensor.transpose(
                    ps_s10[:, kb:kb + 1],
                    nrm[:, kb * 128:(kb + 1) * 128], ident[:1, :1])
            nc.scalar.copy(out=s10T, in_=ps_s10)

            # -- theta chunks --
            def theta_chunk(c):
                h0 = c * 8
                ps_t = ps_misc.tile([CI, 512], F32, tag="m",
                                    name=f"ps_t{c}")
                nc.tensor.matmul(
                    ps_t, r_(thw_sb), r_(xTf[:, c * 512:(c + 1) * 512]),
                    start=True, stop=True)
                t_lin = ttmp.tile([CI, 8, W], F32, tag="tl")
                nc.vector.tensor_scalar_add(
                    t_lin.rearrange("p a b -> p (a b)"), ps_t, thb_sb)
                t_neg = ttmp.tile([CI, 8, W], F32, tag="tn")
                nc.vector.tensor_scalar(
                    t_neg.rearrange("p a b -> p (a b)"),
                    t_lin.rearrange("p a b -> p (a b)"),
                    0.0, tha_sb, Alu.min, Alu.mult)
                nc.vector.scalar_tensor_tensor(
                    out=thetaT_pad[:, 1 + h0:9 + h0, 1:65],
                    in0=t_lin, scalar=0.0, in1=t_neg,
                    op0=Alu.max, op1=Alu.add)

            for c in range(4):
                theta_chunk(c)
            keepalive("t03")

            # -- g path: batched matmuls + prelu per chunk, then DRAM --
            for xc in range(4):
                ps_g8 = ps_misc.tile([128, 512], F32, tag="m",
                                     name=f"ps_g8_{xc}")
                for i, t in enumerate(range(xc * 8, (xc + 1) * 8)):
                    nc.tensor.matmul(
                        ps_g8[:, i * 64:(i + 1) * 64],
                        r_(xTf[:, t * 128:(t + 1) * 128]), r_(gw_sb),
                        start=True, stop=True)
                gv = gtmp.tile([128, 512], F32, tag="gv")
                nc.vector.tensor_add(gv, ps_g8, gb8_bc)
                gm1 = gtmp.tile([128, 512], F32, tag="gm1")
                nc.vector.tensor_scalar_max(gm1, gv, 0.0)
                nc.vector.tensor_scalar_min(gv, gv, 0.0)
                nc.vector.tensor_mul(gv, gv, ga6_8bc)
                nc.vector.scalar_tensor_tensor(
                    out=g_sb[:, xc * 8:(xc + 1) * 8, :].rearrange(
                        "p a b -> p (a b)"),
                    in0=gm1, scalar=1.0 / 6.0, in1=gv,
                    op0=Alu.mult, op1=Alu.add)
            g_int = g_pad[4:68, 4:68, :].rearrange(
                "(t a) w c -> a w t c", a=2)
            for p1 in range(2):
                nc.sync.dma_start(
                    out=g_int[p1], in_=g_sb[p1 * 64:(p1 + 1) * 64, :, :])
            keepalive("g")

            for c in range(4, 8):
                theta_chunk(c)

        # ---- stage 2: fused scores/softmax/deconv pipeline ----
        with ExitStack() as st2:
            e2 = st2.enter_context
            kgp = e2(tc.tile_pool(name="kgp", bufs=1))
            rbp = e2(tc.tile_pool(name="rbp", bufs=3))
            schp = e2(tc.tile_pool(name="schp", bufs=2))
            trp = e2(tc.tile_pool(name="trp", bufs=3))

            # gather the 18 dynamic-filter tiles straight from g_pad
            gp4 = g_pad.rearrange("(hq hr) (wq wr) c -> hq wq hr (wr c)",
                                  hr=4, wr=4)
            kg = {}
            for q in range(3):
                for qw in range(3):
                    for kb in range(2):
                        t_ = kgp.tile([128, 4, 256], BF16,
                                      tag=f"kg{q}{qw}{kb}",
                                      name=f"kg{q}{qw}{kb}")
                        for r in range(4):
                            nc.sync.dma_start(
                                out=t_[:, r, :],
                                in_=gp4[kb * 8 + q: kb * 8 + q + 8,
                                        qw: qw + 16, r, :])
                        kg[(q, qw, kb)] = t_.rearrange("p r x -> p (r x)")

            yr3 = y_h.ap().rearrange(
                "(pc kk pp r) (Mw w) c -> pc pp Mw kk r w c",
                kk=4, pp=2, r=4, w=4)
            pending = [None]

            def drain(pend):
                tr_in, pc, u = pend
                ps_t2 = ps_tr.tile([128, 512], BF16, tag="tt",
                                   name=f"ps_tr{pc}_{u}")
                for k in range(4):
                    nc.tensor.transpose(
                        ps_t2[:, k * 128:(k + 1) * 128],
                        tr_in[:, k * 128:(k + 1) * 128], identB)
                st_ = staging.tile([128, 512], F32, tag="stg",
                                   name=f"st{pc}_{u}")
                nc.scalar.copy(out=st_, in_=ps_t2)
                st3 = st_.rearrange("p (k rw c) -> p k rw c", k=4, rw=2)
                rr = u // 2
                w0 = 2 * (u % 2)
                for p1 in range(2):
                    nc.sync.dma_start(
                        out=yr3[pc, p1, :, :, rr, w0:w0 + 2, :],
                        in_=st3[p1 * 64:(p1 + 1) * 64])

            def deconv_pc(pc):
                h0 = pc * 8
                for u in range(8):
                    ps_o = ps_d.tile([128, 512], F32, tag="d",
                                     name=f"ps_o{pc}_{u}")
                    first = True
                    for q in range(3):
                        for qw in range(3):
                            for kb in range(2):
                                nc.tensor.matmul(
                                    ps_o,
                                    kg[(q, qw, kb)][:, u * 128:
                                                    (u + 1) * 128],
                                    attnT[:, kb, h0 + 2 - q:h0 + 10 - q,
                                          2 - qw:66 - qw],
                                    start=first,
                                    stop=(q == 2 and qw == 2 and kb == 1))
                                first = False
                    tr_in = trp.tile([128, 512], BF16, tag="ti",
                                     name=f"ti{pc}_{u}")
                    nc.scalar.copy(out=tr_in, in_=ps_o)
                    if pending[0] is not None:
                        drain(pending[0])
                    pending[0] = (tr_in, pc, u)

            for ch in range(8):
                h0 = ch * 8
                for kb in range(2):
                    ps_s = ps_sc.tile([128, 512], F32, tag="sc",
                                      name=f"ps_s{ch}_{kb}")
                    first = True
                    for kh in range(3):
                        for kw in range(3):
                            nc.tensor.matmul(
                                ps_s,
                                phi_patchT[:, kh, kw,
                                           kb * 128:(kb + 1) * 128],
                                thetaT_pad[:, h0 + kh:h0 + kh + 8,
                                           kw:kw + 64],
                                start=first, stop=(kh == 2 and kw == 2))
                            first = False
                    nc.scalar.activation(
                        out=attnT[:, kb, 1 + h0:9 + h0, 1:65],
                        in_=ps_s.rearrange("p (a b) -> p a b", b=64),
                        func=Act.Exp, scale=s10T[:, kb:kb + 1])
                if ch >= 2:
                    deconv_pc(ch - 2)
                ps_S = ps_misc.tile([1, 512], F32, tag="m", name=f"ps_S{ch}")
                for kb in range(2):
                    nc.tensor.matmul(
                        ps_S, ones128,
                        attnT[:, kb, 1 + h0:9 + h0, 1:65],
                        start=(kb == 0), stop=(kb == 1))
                sch = schp.tile([1, 512], F32, tag="sch", name=f"sch{ch}")
                nc.vector.reciprocal(sch, ps_S)
                rb_t = rbp.tile([128, 512], F32, tag="rb", name=f"rb{ch}")
                nc.gpsimd.partition_broadcast(rb_t, sch)
                rb3 = rb_t.rearrange("p (a b) -> p a b", b=64)
                for kb in range(2):
                    nc.vector.tensor_mul(
                        attnT[:, kb, 1 + h0:9 + h0, 1:65],
                        attnT[:, kb, 1 + h0:9 + h0, 1:65], rb3)
            deconv_pc(6)
            deconv_pc(7)
            drain(pending[0])

    nc.finalize()
    return nc


def kernel(**inputs):
    from concourse.bass_utils import run_bass_kernel_spmd

    if "nc" not in _CACHE:
        _CACHE["nc"] = _build_nc()
    nc = _CACHE["nc"]

    arrs = {k: np.ascontiguousarray(np.asarray(v, dtype=np.float32))
            for k, v in inputs.items()}
    x = arrs.pop("x")
    in_maps = [dict(arrs, x=x[b]) for b in range(B)]
    res = run_bass_kernel_spmd(nc, in_maps, core_ids=list(range(B)))
    return np.stack([res.results[b]["y"] for b in range(B)])


# revision 10
# speedup vs baseline: 1.1911x; 1.0074x over previous
"""Cross-Scale Non-Local Attention kernel for 8x Trainium2 NeuronCores.

Data-parallel over batch: each of the 8 cores processes one sample
(B=8, H=W=64, C=64). Per-core Bass/Tile program:

  1. x loaded in 4 chunks; PE-transposed to channel-major xT [64, 4096];
     bilinear partials accumulated per chunk on DVE.
  2. phi computed 4-fold replicated on 128 partitions (phi_w stacked 4x)
     so the 3x3 patch taps can be packed 4-per-matmul: stationaries
     phi_pack [128=4taps*32ci, 2grp, 256n] in fp16; L2 norms ->
     s10 = 10/max(norm,1e-6) transposed to [128,2] on the PE.
  3. theta computed 4-fold replicated (theta_w stacked 4x), prelu on
     [128,512]; gpsimd builds per-tap-shifted bf16 views Th_g0/Th_g1
     [128, 64, 64] and Th1 [32, 64, 64] so score matmuls use K=128.
  4. g = prelu(xT.T @ g_w)/6 in bf16 (batched prelu on [128,512]),
     written into the interior of a zero-padded DRAM image g_pad
     [72,72,64]; 18 shifted dynamic-filter views kg[q,qw,kb]
     [n=128, (r rw c)=1024] gathered back by strided DMA.
  5. Per pixel-chunk ch: scoresT [n,pix] via 2 K=128 + 1 K=32 matmuls
     (fp16 phi stationary, bf16 theta moving); E = exp(s10*score);
     S = ones.T @ E; E *= 1/S broadcast.
  6. Deconv as polyphase conv-transpose, one chunk behind the scores:
     psum[rc 128, pix 512] += kg[q,qw,kb][:, u*128:(u+1)*128].T @ E_win
     over 18 shifts; PSUM drained to bf16, PE-transposed in bf16, cast
     to f32, one batched DMA pair per (pc, u).
"""

import numpy as np

_CACHE = {}

B = 8
H = W = 64
C = 64
CI = 32
HS = WS = 16
N = 256
PH = 66          # padded attn spatial extent (64 + 1 halo each side)
GP = 72          # padded g spatial extent (64 + 4 each side)

# tap packing groups: G0 = taps 0-3, G1 = taps 4-7, tap 8 = (2,2) alone
TAPS = [(kh, kw) for kh in range(3) for kw in range(3)]


def _build_nc():
    import concourse.bass as bass
    import concourse.tile as tile
    from concourse import bacc, mybir
    from concourse.masks import make_identity
    from contextlib import ExitStack

    F32 = mybir.dt.float32
    F32R = mybir.dt.float32r
    BF16 = mybir.dt.bfloat16
    F16 = mybir.dt.float16
    Alu = mybir.AluOpType
    Act = mybir.ActivationFunctionType

    def r_(ap):
        return ap.bitcast(F32R)

    nc = bacc.Bacc("TRN2", debug=False)

    x_h = nc.dram_tensor("x", [H, W, C], F32, kind="ExternalInput")
    thw_h = nc.dram_tensor("theta_w", [C, CI], F32, kind="ExternalInput")
    thb_h = nc.dram_tensor("theta_b", [CI], F32, kind="ExternalInput")
    tha_h = nc.dram_tensor("theta_alpha", [CI], F32, kind="ExternalInput")
    phw_h = nc.dram_tensor("phi_w", [C, CI], F32, kind="ExternalInput")
    phb_h = nc.dram_tensor("phi_b", [CI], F32, kind="ExternalInput")
    pha_h = nc.dram_tensor("phi_alpha", [CI], F32, kind="ExternalInput")
    gw_h = nc.dram_tensor("g_w", [C, C], F32, kind="ExternalInput")
    gb_h = nc.dram_tensor("g_b", [C], F32, kind="ExternalInput")
    ga_h = nc.dram_tensor("g_alpha", [C], F32, kind="ExternalInput")
    y_h = nc.dram_tensor("y", [4 * H, 4 * W, C], F32, kind="ExternalOutput")

    with tile.TileContext(nc) as tc, ExitStack() as top:
        ec = top.enter_context

        consts = ec(tc.tile_pool(name="consts", bufs=1))
        xp_pool = ec(tc.tile_pool(name="xp_pool", bufs=1))
        persist = ec(tc.tile_pool(name="persist", bufs=1))
        phip = ec(tc.tile_pool(name="phip", bufs=1))
        dramp = ec(tc.tile_pool(name="dramp", bufs=1, space="DRAM"))
        staging = ec(tc.tile_pool(name="staging", bufs=3))
        ps_misc = ec(tc.tile_pool(name="ps_misc", bufs=2, space="PSUM"))
        ps_sc = ec(tc.tile_pool(name="ps_sc", bufs=2, space="PSUM"))
        ps_d = ec(tc.tile_pool(name="ps_d", bufs=2, space="PSUM"))
        ps_tr = ec(tc.tile_pool(name="ps_tr", bufs=2, space="PSUM"))

        # ---- constants / weights ----
        ident = consts.tile([128, 128], F32)
        make_identity(nc, ident)
        identB = consts.tile([128, 128], BF16)
        nc.vector.tensor_copy(out=identB, in_=ident)
        # HAM warmup + keepalive matmuls
        wu = consts.tile([128, 512], F32)
        nc.vector.memset(wu, 0.0)
        ps_wu = ps_sc.tile([128, 512], F32, tag="sc", name="ps_wu")
        for i in range(8):
            nc.tensor.matmul(ps_wu, wu[:, :128], wu, start=True, stop=True)

        kan = [0]

        def ka_dep(rhs_ap, lhsT_ap):
            # HAM keepalive: junk matmul whose moving operand depends on a
            # freshly produced tile, so it fires exactly when the PE would
            # otherwise go idle waiting on DVE/ACT/DMA.
            kan[0] += 1
            p = rhs_ap.partition_size()
            ps_ka = ps_sc.tile([p, 512], F32, tag="sc", name=f"ka{kan[0]}")
            nc.tensor.matmul(ps_ka[:, :rhs_ap.free_size()],
                             lhsT_ap[:rhs_ap.partition_size(), :p],
                             rhs_ap, start=True, stop=True)

        # x chunks first on the DMA queues
        xP = xp_pool.tile([128, 32, C], F32)
        x_r = x_h.ap().rearrange("h w c -> (h w) c").rearrange(
            "(t p) c -> p t c", p=128)
        for xc in range(4):
            nc.sync.dma_start(
                out=xP[:, xc * 8:(xc + 1) * 8, :],
                in_=x_r[:, xc * 8:(xc + 1) * 8, :])
        thw_sb = consts.tile([C, CI], F32)
        nc.sync.dma_start(out=r_(thw_sb), in_=r_(thw_h.ap()))
        phw_sb = consts.tile([C, CI], F32)
        nc.sync.dma_start(out=r_(phw_sb), in_=r_(phw_h.ap()))
        thb_sb = consts.tile([CI, 1], F32)
        nc.sync.dma_start(out=thb_sb, in_=thb_h.ap().unsqueeze(1))
        tha_sb = consts.tile([CI, 1], F32)
        nc.sync.dma_start(out=tha_sb, in_=tha_h.ap().unsqueeze(1))
        phb_sb = consts.tile([CI, 1], F32)
        nc.sync.dma_start(out=phb_sb, in_=phb_h.ap().unsqueeze(1))
        pha_sb = consts.tile([CI, 1], F32)
        nc.sync.dma_start(out=pha_sb, in_=pha_h.ap().unsqueeze(1))
        gw_sb = consts.tile([C, C], F32)
        nc.sync.dma_start(out=r_(gw_sb), in_=r_(gw_h.ap()))
        gb_row = consts.tile([1, C], F32)
        nc.sync.dma_start(out=gb_row, in_=gb_h.ap().unsqueeze(0))
        ga_row = consts.tile([1, C], F32)
        nc.sync.dma_start(out=ga_row, in_=ga_h.ap().unsqueeze(0))
        gb8_row = consts.tile([1, 8, C], F32)
        ga8_row = consts.tile([1, 8, C], F32)
        for i in range(8):
            nc.vector.tensor_copy(out=gb8_row[:, i, :], in_=gb_row)
            nc.vector.tensor_copy(out=ga8_row[:, i, :], in_=ga_row)
        gb8_bc = consts.tile([128, 512], F32)
        nc.gpsimd.partition_broadcast(
            gb8_bc, gb8_row.rearrange("p a b -> p (a b)"))
        ga6_8bc = consts.tile([128, 512], F32)
        nc.gpsimd.partition_broadcast(
            ga6_8bc, ga8_row.rearrange("p a b -> p (a b)"))
        nc.vector.tensor_scalar_mul(ga6_8bc, ga6_8bc, 1.0 / 6.0)
        z66 = consts.tile([128, PH], F32)
        nc.vector.memset(z66, 0.0)
        o1 = consts.tile([128, 1], F32)
        nc.vector.memset(o1, 1.0)
        ones32 = consts.tile([CI, 1], F32)
        nc.vector.tensor_copy(out=r_(ones32), in_=o1[:CI])
        ones128 = consts.tile([128, 1], BF16)
        nc.vector.tensor_copy(out=ones128, in_=o1)
        s10T = consts.tile([128, 2], F32)

        # ---- persistent activation buffers ----
        thetaT_pad = persist.tile([CI, PH, PH], BF16)
        nc.vector.tensor_copy(out=thetaT_pad[:, 0, :], in_=z66[:CI])
        nc.vector.tensor_copy(out=thetaT_pad[:, PH - 1, :], in_=z66[:CI])
        nc.vector.tensor_copy(out=thetaT_pad[:, :, 0], in_=z66[:CI])
        nc.vector.tensor_copy(out=thetaT_pad[:, :, PH - 1], in_=z66[:CI])
        attnT = persist.tile([128, 2, PH, PH], BF16)
        for kb in range(2):
            nc.vector.tensor_copy(out=attnT[:, kb, 0, :], in_=z66)
            nc.vector.tensor_copy(out=attnT[:, kb, PH - 1, :], in_=z66)
            nc.vector.tensor_copy(out=attnT[:, kb, :, 0], in_=z66)
            nc.vector.tensor_copy(out=attnT[:, kb, :, PH - 1], in_=z66)
        phi_patchT = persist.tile([CI, 3, 3, N], F16)

        phiT_pad = phip.tile([CI, 18, 18], F32)
        nc.vector.memset(phiT_pad, 0.0)
        n2p = phip.tile([1, 324], F32)
        nrm = phip.tile([1, N], F32)
        phi_inT = phip.tile([C, HS, WS], F32)

        # zero-padded g image in DRAM; polyphase kg views are strided reads
        g_pad = dramp.tile([GP, GP, C], BF16)
        ztb = consts.tile([128, 512], BF16)
        nc.vector.memset(ztb, 0.0)
        gpf = g_pad.rearrange("a b c -> (a b c)")
        head = gpf[: 5 * 65536].rearrange("(k p f) -> k p f", p=128, f=512)
        for k in range(5):
            nc.sync.dma_start(out=head[k], in_=ztb)
        tail = gpf[5 * 65536:].rearrange("(p f) -> p f", f=512)
        nc.sync.dma_start(out=tail, in_=ztb[: tail.shape[0], :])

        with ExitStack() as st1:
            e1 = st1.enter_context
            xt_pool = e1(tc.tile_pool(name="xt_pool", bufs=1))
            gsb_pool = e1(tc.tile_pool(name="gsb_pool", bufs=1))
            ttmp = e1(tc.tile_pool(name="ttmp", bufs=2))
            gtmp = e1(tc.tile_pool(name="gtmp", bufs=3))

            xT = xt_pool.tile([C, H, W], F32)
            xTf = xT.rearrange("c h w -> c (h w)")
            g_sb = gsb_pool.tile([128, 32, C], BF16)
            xv = xT.rearrange("c (hq hs) (wq ws) -> c hq hs wq ws",
                              hs=4, ws=4)

            # -- x transposes (4 per PSUM bank) + bilinear per chunk --
            for xc in range(4):
                for t4 in range(2):
                    t0 = xc * 8 + t4 * 4
                    ps_x4 = ps_misc.tile([C, 4, 128], F32, tag="m",
                                         name=f"ps_x4_{xc}_{t4}")
                    for i in range(4):
                        nc.tensor.transpose(
                            ps_x4[:, i, :], xP[:, t0 + i, :], ident)
                    nc.scalar.copy(
                        out=r_(xTf[:, t0 * 128:(t0 + 4) * 128]),
                        in_=ps_x4.rearrange("p a b -> p (a b)"))
                xvc = xv[:, 4 * xc:4 * (xc + 1)]
                pslc = phi_inT[:, 4 * xc:4 * (xc + 1), :]
                nc.vector.tensor_add(r_(pslc), xvc[:, :, 1, :, 1],
                                     xvc[:, :, 1, :, 2])
                nc.vector.tensor_add(r_(pslc), pslc, xvc[:, :, 2, :, 1])
                nc.vector.tensor_add(r_(pslc), pslc, xvc[:, :, 2, :, 2])
                nc.vector.tensor_scalar_mul(r_(pslc), pslc, 0.25)
                ka_dep(r_(xTf[:, xc * 1024:xc * 1024 + 512]), r_(wu))
                ka_dep(r_(phi_inT.rearrange(
                    "c a b -> c (a b)")[:, 4 * xc * WS:(4 * xc + 4) * WS]),
                    r_(wu))

            # -- phi + patches + norms + s10 --
            ps_phi = ps_misc.tile([CI, N], F32, tag="m")
            nc.tensor.matmul(
                ps_phi, r_(phw_sb), r_(phi_inT.rearrange("c a b -> c (a b)")),
                start=True, stop=True)
            p_lin = ttmp.tile([CI, HS, WS], F32, tag="pl")
            nc.vector.tensor_scalar_add(
                p_lin.rearrange("p a b -> p (a b)"), ps_phi, phb_sb)
            p_neg = ttmp.tile([CI, HS, WS], F32, tag="pn")
            nc.vector.tensor_scalar(
                p_neg.rearrange("p a b -> p (a b)"),
                p_lin.rearrange("p a b -> p (a b)"),
                0.0, pha_sb, Alu.min, Alu.mult)
            nc.vector.scalar_tensor_tensor(
                out=phiT_pad[:, 1:17, 1:17],
                in0=p_lin, scalar=0.0, in1=p_neg,
                op0=Alu.max, op1=Alu.add)

            for kh in range(3):
                for kw in range(3):
                    nc.vector.tensor_copy(
                        out=phi_patchT[:, kh, kw, :].rearrange(
                            "p (a b) -> p a b", b=WS),
                        in_=phiT_pad[:, kh:kh + 16, kw:kw + 16])

            sq = ttmp.tile([CI, 324], F32, tag="sq")
            nc.scalar.activation(r_(sq),
                                 phiT_pad.rearrange("p a b -> p (a b)"),
                                 Act.Square)
            ps_n2 = ps_misc.tile([1, 324], F32, tag="m")
            nc.tensor.matmul(ps_n2, r_(ones32), r_(sq), start=True, stop=True)
            nc.scalar.copy(out=n2p, in_=ps_n2)
            n2v = n2p.rearrange("p (a b) -> p a b", b=18)
            nrm3 = nrm.rearrange("p (a b) -> p a b", b=WS)
            nc.vector.tensor_add(nrm3, n2v[:, 0:16, 0:16], n2v[:, 0:16, 1:17])
            for kh in range(3):
                for kw in range(3):
                    if kh == 0 and kw < 2:
                        continue
                    nc.vector.tensor_add(
                        nrm3, nrm3, n2v[:, kh:kh + 16, kw:kw + 16])
            nc.scalar.sqrt(nrm, nrm)
            nc.vector.tensor_scalar_max(nrm, nrm, 1e-6)
            nc.vector.reciprocal(nrm, nrm)
            nc.vector.tensor_scalar_mul(nrm, nrm, 10.0)
            ps_s10 = ps_misc.tile([128, 2], F32, tag="m", name="ps_s10")
            for kb in range(2):
                nc.tensor.transpose(
                    ps_s10[:, kb:kb + 1],
                    nrm[:, kb * 128:(kb + 1) * 128], ident[:1, :1])
            nc.scalar.copy(out=s10T, in_=ps_s10)
            ka_dep(phi_patchT.rearrange("p a b n -> p (a b n)")[:, :512],
                   ztb)

            # -- theta chunks --
            def theta_chunk(c):
                h0 = c * 8
                ps_t = ps_misc.tile([CI, 512], F32, tag="m",
                                    name=f"ps_t{c}")
                nc.tensor.matmul(
                    ps_t, r_(thw_sb), r_(xTf[:, c * 512:(c + 1) * 512]),
                    start=True, stop=True)
                t_lin = ttmp.tile([CI, 8, W], F32, tag="tl")
                nc.vector.tensor_scalar_add(
                    t_lin.rearrange("p a b -> p (a b)"), ps_t, thb_sb)
                t_neg = ttmp.tile([CI, 8, W], F32, tag="tn")
                nc.vector.tensor_scalar(
                    t_neg.rearrange("p a b -> p (a b)"),
                    t_lin.rearrange("p a b -> p (a b)"),
                    0.0, tha_sb, Alu.min, Alu.mult)
                nc.vector.scalar_tensor_tensor(
                    out=thetaT_pad[:, 1 + h0:9 + h0, 1:65],
                    in0=t_lin, scalar=0.0, in1=t_neg,
                    op0=Alu.max, op1=Alu.add)
                ka_dep(thetaT_pad[:, 1 + h0:9 + h0, 1:65], ztb)

            for c in range(4):
                theta_chunk(c)

            # -- g path: batched matmuls + prelu per chunk, then DRAM --
            for xc in range(4):
                ps_g8 = ps_misc.tile([128, 512], F32, tag="m",
                                     name=f"ps_g8_{xc}")
                for i, t in enumerate(range(xc * 8, (xc + 1) * 8)):
                    nc.tensor.matmul(
                        ps_g8[:, i * 64:(i + 1) * 64],
                        r_(xTf[:, t * 128:(t + 1) * 128]), r_(gw_sb),
                        start=True, stop=True)
                gv = gtmp.tile([128, 512], F32, tag="gv")
                nc.vector.tensor_add(gv, ps_g8, gb8_bc)
                gm1 = gtmp.tile([128, 512], F32, tag="gm1")
                nc.vector.tensor_scalar_max(gm1, gv, 0.0)
                nc.vector.tensor_scalar_min(gv, gv, 0.0)
                nc.vector.tensor_mul(gv, gv, ga6_8bc)
                nc.vector.scalar_tensor_tensor(
                    out=g_sb[:, xc * 8:(xc + 1) * 8, :].rearrange(
                        "p a b -> p (a b)"),
                    in0=gm1, scalar=1.0 / 6.0, in1=gv,
                    op0=Alu.mult, op1=Alu.add)
                ka_dep(g_sb[:, xc * 8:(xc + 1) * 8, :].rearrange(
                    "p a b -> p (a b)"), ztb)
            g_int = g_pad[4:68, 4:68, :].rearrange(
                "(t a) w c -> a w t c", a=2)
            for p1 in range(2):
                nc.sync.dma_start(
                    out=g_int[p1], in_=g_sb[p1 * 64:(p1 + 1) * 64, :, :])

            for c in range(4, 8):
                theta_chunk(c)

        # ---- stage 2: fused scores/softmax/deconv pipeline ----
        with ExitStack() as st2:
            e2 = st2.enter_context
            kgp = e2(tc.tile_pool(name="kgp", bufs=1))
            rbp = e2(tc.tile_pool(name="rbp", bufs=3))
            schp = e2(tc.tile_pool(name="schp", bufs=2))
            trp = e2(tc.tile_pool(name="trp", bufs=3))

            # gather the 18 dynamic-filter tiles straight from g_pad
            gp4 = g_pad.rearrange("(hq hr) (wq wr) c -> hq wq hr (wr c)",
                                  hr=4, wr=4)
            kg = {}
            for q in range(3):
                for qw in range(3):
                    for kb in range(2):
                        t_ = kgp.tile([128, 4, 256], BF16,
                                      tag=f"kg{q}{qw}{kb}",
                                      name=f"kg{q}{qw}{kb}")
                        for r in range(4):
                            nc.sync.dma_start(
                                out=t_[:, r, :],
                                in_=gp4[kb * 8 + q: kb * 8 + q + 8,
                                        qw: qw + 16, r, :])
                        kg[(q, qw, kb)] = t_.rearrange("p r x -> p (r x)")

            yr3 = y_h.ap().rearrange(
                "(pc kk pp r) (Mw w) c -> pc pp Mw kk r w c",
                kk=4, pp=2, r=4, w=4)
            pending = [None]

            def drain(pend):
                tr_in, pc, u = pend
                ps_t2 = ps_tr.tile([128, 512], BF16, tag="tt",
                                   name=f"ps_tr{pc}_{u}")
                for k in range(4):
                    nc.tensor.transpose(
                        ps_t2[:, k * 128:(k + 1) * 128],
                        tr_in[:, k * 128:(k + 1) * 128], identB)
                st_ = staging.tile([128, 512], F32, tag="stg",
                                   name=f"st{pc}_{u}")
                nc.scalar.copy(out=st_, in_=ps_t2)
                st3 = st_.rearrange("p (k rw c) -> p k rw c", k=4, rw=2)
                rr = u // 2
                w0 = 2 * (u % 2)
                for p1 in range(2):
                    nc.sync.dma_start(
                        out=yr3[pc, p1, :, :, rr, w0:w0 + 2, :],
                        in_=st3[p1 * 64:(p1 + 1) * 64])

            def deconv_pc(pc):
                h0 = pc * 8
                for u in range(8):
                    ps_o = ps_d.tile([128, 512], F32, tag="d",
                                     name=f"ps_o{pc}_{u}")
                    first = True
                    for q in range(3):
                        for qw in range(3):
                            for kb in range(2):
                                nc.tensor.matmul(
                                    ps_o,
                                    kg[(q, qw, kb)][:, u * 128:
                                                    (u + 1) * 128],
                                    attnT[:, kb, h0 + 2 - q:h0 + 10 - q,
                                          2 - qw:66 - qw],
                                    start=first,
                                    stop=(q == 2 and qw == 2 and kb == 1))
                                first = False
                    tr_in = trp.tile([128, 512], BF16, tag="ti",
                                     name=f"ti{pc}_{u}")
                    nc.scalar.copy(out=tr_in, in_=ps_o)
                    if pending[0] is not None:
                        drain(pending[0])
                    pending[0] = (tr_in, pc, u)

            for ch in range(8):
                h0 = ch * 8
                for kb in range(2):
                    ps_s = ps_sc.tile([128, 512], F32, tag="sc",
                                      name=f"ps_s{ch}_{kb}")
                    first = True
                    for kh in range(3):
                        for kw in range(3):
                            nc.tensor.matmul(
                                ps_s,
                                phi_patchT[:, kh, kw,
                                           kb * 128:(kb + 1) * 128],
                                thetaT_pad[:, h0 + kh:h0 + kh + 8,
                                           kw:kw + 64],
                                start=first, stop=(kh == 2 and kw == 2))
                            first = False
                    nc.scalar.activation(
                        out=attnT[:, kb, 1 + h0:9 + h0, 1:65],
                        in_=ps_s.rearrange("p (a b) -> p a b", b=64),
                        func=Act.Exp, scale=s10T[:, kb:kb + 1])
                if ch >= 2:
                    deconv_pc(ch - 2)
                ps_S = ps_misc.tile([1, 512], F32, tag="m", name=f"ps_S{ch}")
                for kb in range(2):
                    nc.tensor.matmul(
                        ps_S, ones128,
                        attnT[:, kb, 1 + h0:9 + h0, 1:65],
                        start=(kb == 0), stop=(kb == 1))
                sch = schp.tile([1, 512], F32, tag="sch", name=f"sch{ch}")
                nc.vector.reciprocal(sch, ps_S)
                rb_t = rbp.tile([128, 512], F32, tag="rb", name=f"rb{ch}")
                nc.gpsimd.partition_broadcast(rb_t, sch)
                rb3 = rb_t.rearrange("p (a b) -> p a b", b=64)
                for kb in range(2):
                    nc.vector.tensor_mul(
                        attnT[:, kb, 1 + h0:9 + h0, 1:65],
                        attnT[:, kb, 1 + h0:9 + h0, 1:65], rb3)
            deconv_pc(6)
            deconv_pc(7)
            drain(pending[0])

    nc.finalize()
    return nc


def kernel(**inputs):
    from concourse.bass_utils import run_bass_kernel_spmd

    if "nc" not in _CACHE:
        _CACHE["nc"] = _build_nc()
    nc = _CACHE["nc"]

    arrs = {k: np.ascontiguousarray(np.asarray(v, dtype=np.float32))
            for k, v in inputs.items()}
    x = arrs.pop("x")
    in_maps = [dict(arrs, x=x[b]) for b in range(B)]
    res = run_bass_kernel_spmd(nc, in_maps, core_ids=list(range(B)))
    return np.stack([res.results[b]["y"] for b in range(B)])
